# revision 15
# baseline (speedup 1.0000x reference)
"""Causal multi-head attention block (B=4, T=2048, C=1024, H=16, D=64) on 8 trn2 cores.

Sharding: core c -> (batch b = c//2, head-group g = c%2 covering heads 8g..8g+8).
Each core computes the qkv projection for its batch restricted to its 8 heads and
flash-style causal attention in transposed orientation. The two cores of a batch
then exchange attention outputs with a per-block fp16 AllGather (0.5 MB per block,
vs an 8 MB fp32 ReduceScatter of y-partials), after which each core runs the full
16-head output projection for all tokens into its own 512 w_proj columns
(w_proj is column-sliced host-side), so no cross-core reduction is needed.

The qkv projection is fused into the attention block loop: projection groups
(8 accumulating matmuls + one PSUM-to-SBUF add) are emitted as filler jobs
between attention pairs, sharing one PSUM accumulator tag with the output-proj
jobs. The ScalarE exp stream paces attention, so the projection matmuls ride in
the PE gaps and keep the HAM clock warm; block n+1's q/k/v is produced during
block n's attention.

Attention is causally trimmed: diagonal 128x512 tiles only compute scores/exp/PV
for live queries (N = 512-off, head B bank-anchored at column 512), with a
[128,128] triangle mask multiply. Softmax denominators come from a ones-column
appended to v (M=65 PV matmuls); normalization is reciprocal + gpsimd
partition_broadcast + one multiply. (A K=1 broadcast matmul into the freed PV
bank would keep GpSimd collective-only, but it produced NaNs on hardware while
passing CoreSim, so the gpsimd broadcast stays; its occasional queueing behind
an AllGather costs ~8us once per run.)
"""
import sys

sys.path.insert(0, '/opt/trn_rl_repo')

from contextlib import ExitStack

import numpy as np

import concourse.bass as bass
import concourse.mybir as mybir
import concourse.tile as tile
from concourse import bacc
from concourse.bass_utils import run_bass_kernel_spmd

B, T, C = 4, 2048, 1024
H, D = 16, 64
HL = H // 2            # heads per core
NP = HL // 2           # head pairs per core
KC = C // 128          # contraction chunks for qkv projection
NT1 = T // 512         # 512-wide query blocks
NT2 = T // 128         # 128-tall key tiles
F32 = mybir.dt.float32
F16 = mybir.dt.float16
EXP = mybir.ActivationFunctionType.Exp

_cached = {}


def install_profile_hook():
    """The agent image's antenv lacks axon_hooks; synthesize it so
    run_bass_kernel_spmd(trace=True) can capture NTFF profiles."""
    import types
    if 'antenv.axon_hooks' in sys.modules:
        return
    mod = types.ModuleType('antenv.axon_hooks')
    mod._hook = None

    def set_axon_ntff_profile_hook(h):
        mod._hook = h

    def get_axon_ntff_profile_hook():
        return mod._hook

    mod.set_axon_ntff_profile_hook = set_axon_ntff_profile_hook
    mod.get_axon_ntff_profile_hook = get_axon_ntff_profile_hook
    sys.modules['antenv.axon_hooks'] = mod
    try:
        from trn_agent_boot.trn_boot import _ntff_profile_via_ctypes
        set_axon_ntff_profile_hook(_ntff_profile_via_ctypes('/opt/axon/libaxon_pjrt.so'))
    except Exception as e:
        print(f"profile hook install failed: {e}", file=sys.stderr)


def build_kernel():
    if 'nc' in _cached:
        return _cached['nc']
    nc = bacc.Bacc("TRN2", target_bir_lowering=False, debug=False, num_devices=8)

    xT = nc.declare_dram_parameter("xT", [C, T], F16, isOutput=False)
    w_qk = nc.declare_dram_parameter("w_qk", [C, 2 * HL * D], F16, isOutput=False)
    w_v = nc.declare_dram_parameter("w_v", [C, HL * D], F16, isOutput=False)
    b_qk = nc.declare_dram_parameter("b_qk", [2 * HL * D, 1], F32, isOutput=False)
    b_v = nc.declare_dram_parameter("b_v", [1, HL * D], F16, isOutput=False)
    w_proj = nc.declare_dram_parameter("w_proj", [C, C // 2], F16, isOutput=False)
    b_proj = nc.declare_dram_parameter("b_proj", [1, C // 2], F16, isOutput=False)
    y_own = nc.declare_dram_parameter("y_own", [T, C // 2], F32, isOutput=True)

    with tile.TileContext(nc) as tc, ExitStack() as st:
        cpool = st.enter_context(tc.tile_pool(name="const", bufs=1))
        v_pool = st.enter_context(tc.tile_pool(name="vstore", bufs=1))
        qk_pool = st.enter_context(tc.tile_pool(name="qkT", bufs=1))
        o_pool = st.enter_context(tc.tile_pool(name="outT", bufs=1, side="right"))
        wppool = st.enter_context(tc.tile_pool(name="wproj", bufs=1))
        po_pool = st.enter_context(tc.tile_pool(name="po", bufs=1))
        xpool = st.enter_context(tc.tile_pool(name="xT", bufs=1))
        wpool = st.enter_context(tc.tile_pool(name="wqk", bufs=1))
        ppool = st.enter_context(tc.tile_pool(name="ptile", bufs=6))
        ypool = st.enter_context(tc.tile_pool(name="ytile", bufs=4))
        sps = st.enter_context(tc.tile_pool(name="s_ps", bufs=1, space="PSUM"))
        pvps = st.enter_context(tc.tile_pool(name="pv_ps", bufs=1, space="PSUM"))
        yps = st.enter_context(tc.tile_pool(name="y_ps", bufs=1, space="PSUM"))
        dram = st.enter_context(tc.tile_pool(name="dram", bufs=1, space="DRAM"))

        # ---- constants ----
        ones128h = cpool.tile([1, 128], F16)
        nc.gpsimd.memset(ones128h[:], 1.0)
        ones_p = cpool.tile([128, HL], F16)
        nc.gpsimd.memset(ones_p[:], 1.0)
        bqk_sb = cpool.tile([128, 2 * NP, 1], F32)
        nc.sync.dma_start(bqk_sb[:], b_qk[:].rearrange("(c p) o -> p c o", p=128))
        # triangle mask: tri01[a, q] = 1 iff q >= a (keep), 0 above-diagonal keys
        tri01 = cpool.tile([128, 128], F16)
        nc.gpsimd.memset(tri01[:], 1.0)
        nc.gpsimd.affine_select(
            out=tri01[:], in_=tri01[:], compare_op=mybir.AluOpType.is_ge,
            fill=0.0, base=0, pattern=[[1, 128]], channel_multiplier=-1,
        )

        # persistent activations
        vst = [v_pool.tile([128, HL, D + 1], F16, tag=f"vs{m}", name=f"vs{m}")
               for m in range(NT2)]
        qkT = [qk_pool.tile([128, T], F16, tag=f"qk{j}", name=f"qk{j}")
               for j in range(2 * NP)]
        outT = [o_pool.tile([128, T], F16, tag=f"o{j}", name=f"o{j}")
                for j in range(NP)]
        wp_sb = [wppool.tile([128, C // 2], F16, tag=f"wp{j}", name=f"wp{j}")
                 for j in range(KC)]
        bp_sb = cpool.tile([1, C // 2], F16)
        nc.sync.dma_start(bp_sb[:], b_proj[:])

        # ---- input DMAs: tiny bias tensors first (they gate the first yps-tag
        # matmuls), then the big weight/activation streams
        xTt = [xpool.tile([128, T], F16, tag=f"x{kc}", name=f"x{kc}")
               for kc in range(KC)]
        wqk_sb = [wpool.tile([128, 2 * HL * D], F16, tag=f"w{kc}", name=f"w{kc}")
                  for kc in range(KC)]
        wv_sb = [wpool.tile([128, HL * D], F16, tag=f"wv{kc}", name=f"wv{kc}")
                 for kc in range(KC)]
        bv_sb = wpool.tile([1, HL * D], F16, tag="bv_sb")
        nc.sync.dma_start(bv_sb[:], b_v[:])
        for kc in range(KC):
            nc.sync.dma_start(wqk_sb[kc][:], w_qk[bass.ts(kc, 128), :])
            nc.scalar.dma_start(xTt[kc][:], xT[bass.ts(kc, 128), :])
        for kc in range(KC):
            nc.scalar.dma_start(wv_sb[kc][:], w_v[bass.ts(kc, 128), :])
        for jj in range(KC):
            nc.scalar.dma_start(wp_sb[jj][:], w_proj[bass.ts(jj, 128), :])

        # bias broadcast rows
        bvb_ps = yps.tile([128, HL * D], F32, tag="yps", bufs=2)
        nc.tensor.matmul(bvb_ps[:], ones128h[:], bv_sb[:], start=True, stop=True)
        bvb = wpool.tile([128, HL, D], F32, tag="bvb")
        nc.vector.tensor_copy(bvb[:], bvb_ps[:].rearrange("p (h d) -> p h d", h=HL))
        bpb = cpool.tile([128, C // 2], F32)
        bpb_ps = yps.tile([128, 512], F32, tag="yps", bufs=2)
        nc.tensor.matmul(bpb_ps[:], ones128h[:], bp_sb[:], start=True, stop=True)
        nc.vector.tensor_copy(bpb[:], bpb_ps[:])

        ag_in = [[dram.tile([512, 256], F16, tag=f"agi{b}h{h}", name=f"agi{b}h{h}")
                  for h in range(2)] for b in range(NT1)]
        ag_out = [[dram.tile([2, 512, 256], F16, tag=f"ago{b}h{h}",
                             name=f"ago{b}h{h}") for h in range(2)]
                  for b in range(NT1)]
        po = {}

        # ---- filler jobs: qkv projection groups / output projection ----
        def qk_job(oc, n):
            ps = yps.tile([128, 512], F32, tag="yps", bufs=2)
            for kc in range(KC):
                nc.tensor.matmul(
                    ps[:], wqk_sb[kc][:, bass.ts(oc, 128)],
                    xTt[kc][:, bass.ts(n, 512)],
                    start=(kc == 0), stop=(kc == KC - 1))
            nc.vector.tensor_scalar_add(
                qkT[oc][:, bass.ts(n, 512)], ps[:], bqk_sb[:, oc, :])

        def v_job(m):
            ps = yps.tile([128, 512], F32, tag="yps", bufs=2)
            for kc in range(KC):
                nc.tensor.matmul(
                    ps[:], xTt[kc][:, bass.ts(m, 128)], wv_sb[kc][:],
                    start=(kc == 0), stop=(kc == KC - 1))
            nc.vector.tensor_add(
                vst[m][:, :, 0:D],
                ps[:].rearrange("p (h d) -> p h d", h=HL), bvb[:])
            nc.vector.tensor_copy(vst[m][:, :, D], ones_p[:])

        def proj_job(blk, mt):
            ps = yps.tile([128, 512], F32, tag="yps", bufs=2)
            for jj in range(KC):
                nc.tensor.matmul(
                    ps[:], po[blk][mt // 2][jj][:, bass.ts(mt % 2, 128)],
                    wp_sb[jj][:],
                    start=(jj == 0), stop=(jj == KC - 1))
            yt = ypool.tile([128, 512], F32, tag="yt")
            nc.vector.tensor_add(yt[:], ps[:], bpb[:])
            nc.sync.dma_start(
                y_own[bass.ds(blk * 512 + mt * 128, 128), :], yt[:])

        def stage_a_jobs(n):
            jobs = []
            for j in range(NP):
                jobs.append((qk_job, (j, n)))
                jobs.append((qk_job, (NP + j, n)))
            for m in range(4 * n, 4 * n + 4):
                jobs.append((v_job, (m,)))
            return jobs

        def emit_exchange(blk):
            # stage this block's attention outputs and AllGather with the peer
            # in two column-halves: ag_out[h] = [both groups' 512 channels,
            # 256 cols]; proj jobs for the first half start while the second
            # half is still on the wire.
            po[blk] = []
            for h in range(2):
                for j in range(NP):
                    nc.sync.dma_start(
                        ag_in[blk][h][bass.ts(j, 128), :],
                        outT[j][:, bass.ds(blk * 512 + h * 256, 256)])
                nc.gpsimd.collective_compute(
                    "AllGather", mybir.AluOpType.bypass,
                    replica_groups=[[0, 1], [2, 3], [4, 5], [6, 7]],
                    ins=[ag_in[blk][h][:].opt()],
                    outs=[ag_out[blk][h][:].opt()],
                )
                out_flat = ag_out[blk][h][:].rearrange("g c q -> (g c) q")
                tiles = []
                for jj in range(KC):
                    t = po_pool.tile([128, 256], F16, tag=f"po{jj}h{h}", bufs=2,
                                     name=f"po{jj}h{h}_{blk}")
                    nc.sync.dma_start(t[:], out_flat[bass.ts(jj, 128), :])
                    tiles.append(t)
                po[blk].append(tiles)

        # ---- fused pipeline over query blocks ----
        for fn, args in stage_a_jobs(0):
            fn(*args)
        pending = []
        for blk in range(NT1):
            nt2 = 4 * (blk + 1)
            if blk + 1 < NT1:
                pending.extend(stage_a_jobs(blk + 1))
            njobs, total_tiles, tiles_done, jobs_done = \
                len(pending), NP * nt2, 0, 0
            for j in range(NP):
                q_t, k_t = qkT[j], qkT[NP + j]
                pv1 = pvps.tile([D + 1, 512], F32, tag="pvA", bufs=1)
                pv2 = pvps.tile([D + 1, 512], F32, tag="pvB", bufs=1)
                for i in range(nt2):
                    t2 = bass.ds(i * 128, 128)
                    off = i * 128 - blk * 512
                    w = 512 - max(off, 0)        # live query columns
                    t1 = bass.ds(blk * 512 + (512 - w), w)
                    # head A scores at cols [0:w], head B anchored at 512
                    # (PSUM-bank aligned); the gap [w:512] is never read.
                    sAB = sps.tile([128, 1024], F32, tag="sAB", bufs=2)
                    nc.tensor.matmul(sAB[:, 0:w], k_t[0:64, t2], q_t[0:64, t1],
                                     start=True, stop=True, tile_position=(0, 0))
                    nc.tensor.matmul(sAB[:, 512:512 + w], k_t[64:128, t2],
                                     q_t[64:128, t1],
                                     start=True, stop=True, tile_position=(64, 0))
                    pAB = ppool.tile([128, 1024], F16, tag="pAB", bufs=6)
                    if w == 512:
                        nc.scalar.activation(pAB[:], sAB[:], EXP, scale=0.125)
                    else:
                        nc.scalar.activation(pAB[:, 0:w], sAB[:, 0:w],
                                             EXP, scale=0.125)
                        nc.scalar.activation(pAB[:, 512:512 + w],
                                             sAB[:, 512:512 + w],
                                             EXP, scale=0.125)
                    if off >= 0:
                        nc.vector.tensor_mul(pAB[:, 0:128], pAB[:, 0:128],
                                             tri01[:])
                        nc.vector.tensor_mul(pAB[:, 512:640],
                                             pAB[:, 512:640], tri01[:])
                    pvs = bass.ds(512 - w, w)
                    nc.tensor.matmul(pv1[:, pvs], vst[i][:, 2 * j, :],
                                     pAB[:, 0:w],
                                     start=(i == 0), stop=(i == nt2 - 1))
                    nc.tensor.matmul(pv2[:, pvs], vst[i][:, 2 * j + 1, :],
                                     pAB[:, 512:512 + w],
                                     start=(i == 0), stop=(i == nt2 - 1))
                    # drip filler jobs into the PE gaps of the ACT-gated
                    # attention stream, spread evenly across the block
                    tiles_done += 1
                    while (pending and
                           jobs_done < (tiles_done * njobs) // total_tiles):
                        fn, args = pending.pop(0)
                        fn(*args)
                        jobs_done += 1
                tq = bass.ds(blk * 512, 512)
                for h, pv in ((0, pv1), (1, pv2)):
                    rs_sb = ppool.tile([1, 512], F32, tag="rs_sb", bufs=3)
                    nc.vector.tensor_copy(rs_sb[:], pv[D:D + 1, :])
                    rec = ppool.tile([1, 512], F32, tag="rec", bufs=3)
                    nc.vector.reciprocal_approx_fast(rec[:], rs_sb[:])
                    rb = ppool.tile([64, 512], F32, tag="rb", bufs=3)
                    nc.gpsimd.partition_broadcast(rb[:], rec[:])
                    nc.vector.tensor_mul(outT[j][h * 64:(h + 1) * 64, tq],
                                         pv[0:D, :], rb[:])
            while pending:
                fn, args = pending.pop(0)
                fn(*args)
            emit_exchange(blk)
            pending = [(proj_job, (blk, mt)) for mt in range(4)]
        while pending:
            fn, args = pending.pop(0)
            fn(*args)

    nc.compile()
    _cached['nc'] = nc
    return nc


def make_in_maps(x, w_qkv, b_qkv, w_proj, b_proj):
    x = np.asarray(x, dtype=np.float32)
    w_qkv = np.asarray(w_qkv, dtype=np.float32)
    b_qkv = np.asarray(b_qkv, dtype=np.float32)
    w_proj = np.asarray(w_proj, dtype=np.float32)
    b_proj = np.asarray(b_proj, dtype=np.float32)

    in_maps = []
    for c in range(8):
        b, g = c // 2, c % 2
        heads = list(range(g * HL, (g + 1) * HL))
        # paired column order: chunk j = [q(h_{2j}) | q(h_{2j+1})], then k chunks
        qcols, kcols = [], []
        for j in range(NP):
            for h in (heads[2 * j], heads[2 * j + 1]):
                qcols.extend(range(h * D, (h + 1) * D))
                kcols.extend(range(C + h * D, C + (h + 1) * D))
        vcols = [2 * C + h * D + d for h in heads for d in range(D)]
        qk_idx = np.array(qcols + kcols)
        v_idx = np.array(vcols)

        in_maps.append({
            "xT": np.ascontiguousarray(x[b].T.astype(np.float16)),
            "w_qk": np.ascontiguousarray(w_qkv[:, qk_idx].astype(np.float16)),
            "w_v": np.ascontiguousarray(w_qkv[:, v_idx].astype(np.float16)),
            "b_qk": np.ascontiguousarray(b_qkv[qk_idx][:, None]),
            "b_v": np.ascontiguousarray(b_qkv[v_idx][None, :].astype(np.float16)),
            "w_proj": np.ascontiguousarray(
                w_proj[:, g * 512:(g + 1) * 512].astype(np.float16)),
            "b_proj": np.ascontiguousarray(
                b_proj[None, g * 512:(g + 1) * 512].astype(np.float16)),
        })
    return in_maps


def run(inputs, trace=False):
    if trace:
        install_profile_hook()
    nc = build_kernel()
    in_maps = make_in_maps(**inputs)
    res = run_bass_kernel_spmd(nc, in_maps, list(range(8)), trace=trace)
    out = np.empty((B, T, C), dtype=np.float32)
    for c in range(8):
        b, g = c // 2, c % 2
        out[b, :, g * 512:(g + 1) * 512] = res.results[c]["y_own"]
    return out, res


def kernel(**inputs) -> np.ndarray:
    out, _ = run(inputs, trace=False)
    return out


# revision 18
# speedup vs baseline: 1.2240x; 1.2240x over previous
"""Causal multi-head attention block (B=4, T=2048, C=1024, H=16, D=64) on 8 trn2 cores.

Sharding: core c -> (batch b = c//2, head-group g = c%2 covering heads 8g..8g+8).
Each core computes the qkv projection for its batch restricted to its 8 heads and
flash-style causal attention in transposed orientation. The two cores of a batch
then exchange attention outputs with a per-block fp16 AllGather (0.5 MB per block,
vs an 8 MB fp32 ReduceScatter of y-partials), after which each core runs the full
16-head output projection for all tokens into its own 512 w_proj columns
(w_proj is column-sliced host-side), so no cross-core reduction is needed.

The qkv projection is fused into the attention block loop: projection groups
(8 accumulating matmuls + one PSUM-to-SBUF add) are emitted as filler jobs
between attention pairs, sharing one PSUM accumulator tag with the output-proj
jobs. The ScalarE exp stream paces attention, so the projection matmuls ride in
the PE gaps and keep the HAM clock warm; block n+1's q/k/v is produced during
block n's attention.

Attention is causally trimmed: diagonal 128x512 tiles only compute scores/exp/PV
for live queries (N = 512-off, head B bank-anchored at column 512), with a
[128,128] triangle mask multiply. Softmax denominators come from a ones-column
appended to v (M=65 PV matmuls); normalization is reciprocal + gpsimd
partition_broadcast + one multiply. (A K=1 broadcast matmul into the freed PV
bank would keep GpSimd collective-only, but it produced NaNs on hardware while
passing CoreSim, so the gpsimd broadcast stays; its occasional queueing behind
an AllGather costs ~8us once per run.)
"""
import sys

sys.path.insert(0, '/opt/trn_rl_repo')

from contextlib import ExitStack

import numpy as np

import concourse.bass as bass
import concourse.mybir as mybir
import concourse.tile as tile
from concourse import bacc
from concourse.bass_utils import run_bass_kernel_spmd

B, T, C = 4, 2048, 1024
H, D = 16, 64
HL = H // 2            # heads per core
NP = HL // 2           # head pairs per core
KC = C // 128          # contraction chunks for qkv projection
NT1 = T // 512         # 512-wide query blocks
NT2 = T // 128         # 128-tall key tiles
F32 = mybir.dt.float32
F16 = mybir.dt.float16
EXP = mybir.ActivationFunctionType.Exp

_cached = {}


def install_profile_hook():
    """The agent image's antenv lacks axon_hooks; synthesize it so
    run_bass_kernel_spmd(trace=True) can capture NTFF profiles."""
    import types
    if 'antenv.axon_hooks' in sys.modules:
        return
    mod = types.ModuleType('antenv.axon_hooks')
    mod._hook = None

    def set_axon_ntff_profile_hook(h):
        mod._hook = h

    def get_axon_ntff_profile_hook():
        return mod._hook

    mod.set_axon_ntff_profile_hook = set_axon_ntff_profile_hook
    mod.get_axon_ntff_profile_hook = get_axon_ntff_profile_hook
    sys.modules['antenv.axon_hooks'] = mod
    try:
        from trn_agent_boot.trn_boot import _ntff_profile_via_ctypes
        set_axon_ntff_profile_hook(_ntff_profile_via_ctypes('/opt/axon/libaxon_pjrt.so'))
    except Exception as e:
        print(f"profile hook install failed: {e}", file=sys.stderr)


def build_kernel():
    if 'nc' in _cached:
        return _cached['nc']
    nc = bacc.Bacc("TRN2", target_bir_lowering=False, debug=False, num_devices=8)

    xT = nc.declare_dram_parameter("xT", [C, T], F16, isOutput=False)
    w_qk = nc.declare_dram_parameter("w_qk", [C, 2 * HL * D], F16, isOutput=False)
    w_v = nc.declare_dram_parameter("w_v", [C, HL * D], F16, isOutput=False)
    b_qk = nc.declare_dram_parameter("b_qk", [2 * HL * D, 1], F32, isOutput=False)
    b_v = nc.declare_dram_parameter("b_v", [1, HL * D], F16, isOutput=False)
    w_proj = nc.declare_dram_parameter("w_proj", [C, C // 2], F16, isOutput=False)
    b_proj = nc.declare_dram_parameter("b_proj", [1, C // 2], F16, isOutput=False)
    y_own = nc.declare_dram_parameter("y_own", [T, C // 2], F32, isOutput=True)

    with tile.TileContext(nc) as tc, ExitStack() as st:
        cpool = st.enter_context(tc.tile_pool(name="const", bufs=1))
        v_pool = st.enter_context(tc.tile_pool(name="vstore", bufs=1))
        qk_pool = st.enter_context(tc.tile_pool(name="qkT", bufs=1))
        o_pool = st.enter_context(tc.tile_pool(name="outT", bufs=1, side="right"))
        wppool = st.enter_context(tc.tile_pool(name="wproj", bufs=1))
        po_pool = st.enter_context(tc.tile_pool(name="po", bufs=1))
        xpool = st.enter_context(tc.tile_pool(name="xT", bufs=1))
        wpool = st.enter_context(tc.tile_pool(name="wqk", bufs=1))
        ppool = st.enter_context(tc.tile_pool(name="ptile", bufs=6))
        ypool = st.enter_context(tc.tile_pool(name="ytile", bufs=4))
        sps = st.enter_context(tc.tile_pool(name="s_ps", bufs=1, space="PSUM"))
        pvps = st.enter_context(tc.tile_pool(name="pv_ps", bufs=1, space="PSUM"))
        yps = st.enter_context(tc.tile_pool(name="y_ps", bufs=1, space="PSUM"))
        dram = st.enter_context(tc.tile_pool(name="dram", bufs=1, space="DRAM"))

        # ---- constants ----
        ones128h = cpool.tile([1, 128], F16)
        nc.gpsimd.memset(ones128h[:], 1.0)
        ones_p = cpool.tile([128, HL], F16)
        nc.gpsimd.memset(ones_p[:], 1.0)
        bqk_sb = cpool.tile([128, 2 * NP, 1], F32)
        nc.sync.dma_start(bqk_sb[:], b_qk[:].rearrange("(c p) o -> p c o", p=128))
        # triangle mask: tri01[a, q] = 1 iff q >= a (keep), 0 above-diagonal keys
        tri01 = cpool.tile([128, 128], F16)
        nc.gpsimd.memset(tri01[:], 1.0)
        nc.gpsimd.affine_select(
            out=tri01[:], in_=tri01[:], compare_op=mybir.AluOpType.is_ge,
            fill=0.0, base=0, pattern=[[1, 128]], channel_multiplier=-1,
        )

        # persistent activations
        vst = [v_pool.tile([128, HL, D + 1], F16, tag=f"vs{m}", name=f"vs{m}")
               for m in range(NT2)]
        qkT = [qk_pool.tile([128, T], F16, tag=f"qk{j}", name=f"qk{j}")
               for j in range(2 * NP)]
        outT = [o_pool.tile([128, T], F16, tag=f"o{j}", name=f"o{j}")
                for j in range(NP)]
        wp_sb = [wppool.tile([128, C // 2], F16, tag=f"wp{j}", name=f"wp{j}")
                 for j in range(KC)]
        bp_sb = cpool.tile([1, C // 2], F16)
        nc.sync.dma_start(bp_sb[:], b_proj[:])

        # ---- input DMAs: tiny bias tensors first (they gate the first yps-tag
        # matmuls), then the big weight/activation streams
        xTt = [xpool.tile([128, T], F16, tag=f"x{kc}", name=f"x{kc}")
               for kc in range(KC)]
        wqk_sb = [wpool.tile([128, 2 * HL * D], F16, tag=f"w{kc}", name=f"w{kc}")
                  for kc in range(KC)]
        wv_sb = [wpool.tile([128, HL * D], F16, tag=f"wv{kc}", name=f"wv{kc}")
                 for kc in range(KC)]
        bv_sb = wpool.tile([1, HL * D], F16, tag="bv_sb")
        nc.sync.dma_start(bv_sb[:], b_v[:])
        for kc in range(KC):
            nc.sync.dma_start(wqk_sb[kc][:], w_qk[bass.ts(kc, 128), :])
            nc.scalar.dma_start(xTt[kc][:], xT[bass.ts(kc, 128), :])
        for kc in range(KC):
            nc.sync.dma_start(wv_sb[kc][:], w_v[bass.ts(kc, 128), :])
        for jj in range(KC):
            nc.sync.dma_start(wp_sb[jj][:], w_proj[bass.ts(jj, 128), :])

        # bias broadcast rows
        bvb_ps = yps.tile([128, HL * D], F32, tag="yps", bufs=2)
        nc.tensor.matmul(bvb_ps[:], ones128h[:], bv_sb[:], start=True, stop=True)
        bvb = wpool.tile([128, HL, D], F32, tag="bvb")
        nc.vector.tensor_copy(bvb[:], bvb_ps[:].rearrange("p (h d) -> p h d", h=HL))
        bpb = cpool.tile([128, C // 2], F32)
        bpb_ps = yps.tile([128, 512], F32, tag="yps", bufs=2)
        nc.tensor.matmul(bpb_ps[:], ones128h[:], bp_sb[:], start=True, stop=True)
        nc.vector.tensor_copy(bpb[:], bpb_ps[:])

        ag_in = [dram.tile([512, 512], F16, tag=f"agi{b}", name=f"agi{b}")
                 for b in range(NT1)]
        ag_out = [dram.tile([2, 512, 512], F16, tag=f"ago{b}", name=f"ago{b}")
                  for b in range(NT1)]
        po = {}

        # ---- filler jobs: qkv projection groups / output projection ----
        def qk_job(oc, n):
            ps = yps.tile([128, 512], F32, tag="yps", bufs=2)
            for kc in range(KC):
                nc.tensor.matmul(
                    ps[:], wqk_sb[kc][:, bass.ts(oc, 128)],
                    xTt[kc][:, bass.ts(n, 512)],
                    start=(kc == 0), stop=(kc == KC - 1))
            nc.vector.tensor_scalar_add(
                qkT[oc][:, bass.ts(n, 512)], ps[:], bqk_sb[:, oc, :])

        def v_job(m):
            ps = yps.tile([128, 512], F32, tag="yps", bufs=2)
            for kc in range(KC):
                nc.tensor.matmul(
                    ps[:], xTt[kc][:, bass.ts(m, 128)], wv_sb[kc][:],
                    start=(kc == 0), stop=(kc == KC - 1))
            nc.vector.tensor_add(
                vst[m][:, :, 0:D],
                ps[:].rearrange("p (h d) -> p h d", h=HL), bvb[:])
            nc.vector.tensor_copy(vst[m][:, :, D], ones_p[:])

        def proj_job(blk, mt):
            ps = yps.tile([128, 512], F32, tag="yps", bufs=2)
            for jj in range(KC):
                nc.tensor.matmul(
                    ps[:], po[blk][jj][:, bass.ts(mt, 128)],
                    wp_sb[jj][:],
                    start=(jj == 0), stop=(jj == KC - 1))
            yt = ypool.tile([128, 512], F32, tag="yt")
            nc.vector.tensor_add(yt[:], ps[:], bpb[:])
            nc.sync.dma_start(
                y_own[bass.ds(blk * 512 + mt * 128, 128), :], yt[:])

        def stage_a_jobs(n):
            jobs = []
            for j in range(NP):
                jobs.append((qk_job, (j, n)))
                jobs.append((qk_job, (NP + j, n)))
            for m in range(4 * n, 4 * n + 4):
                jobs.append((v_job, (m,)))
            return jobs

        def emit_exchange(blk):
            # stage this block's attention outputs and AllGather with the peer:
            # ag_out = [both groups' 512 channels, this block's 512 cols].
            for j in range(NP):
                nc.sync.dma_start(
                    ag_in[blk][bass.ts(j, 128), :],
                    outT[j][:, bass.ds(blk * 512, 512)])
            nc.gpsimd.collective_compute(
                "AllGather", mybir.AluOpType.bypass,
                replica_groups=[[0, 1], [2, 3], [4, 5], [6, 7]],
                ins=[ag_in[blk][:].opt()],
                outs=[ag_out[blk][:].opt()],
            )
            out_flat = ag_out[blk][:].rearrange("g c q -> (g c) q")
            tiles = []
            for jj in range(KC):
                t = po_pool.tile([128, 512], F16, tag=f"po{jj}", bufs=2,
                                 name=f"po{jj}_{blk}")
                nc.sync.dma_start(t[:], out_flat[bass.ts(jj, 128), :])
                tiles.append(t)
            po[blk] = tiles

        # ---- fused pipeline over query blocks ----
        for fn, args in stage_a_jobs(0):
            fn(*args)
        pending = []
        for blk in range(NT1):
            nt2 = 4 * (blk + 1)
            if blk + 1 < NT1:
                pending.extend(stage_a_jobs(blk + 1))
            for j in range(NP):
                q_t, k_t = qkT[j], qkT[NP + j]
                pv1 = pvps.tile([D + 1, 512], F32, tag="pvA", bufs=1)
                pv2 = pvps.tile([D + 1, 512], F32, tag="pvB", bufs=1)
                for i in range(nt2):
                    t2 = bass.ds(i * 128, 128)
                    off = i * 128 - blk * 512
                    w = 512 - max(off, 0)        # live query columns
                    t1 = bass.ds(blk * 512 + (512 - w), w)
                    # head A scores at cols [0:w], head B anchored at 512
                    # (PSUM-bank aligned); the gap [w:512] is never read.
                    sAB = sps.tile([128, 1024], F32, tag="sAB", bufs=2)
                    nc.tensor.matmul(sAB[:, 0:w], k_t[0:64, t2], q_t[0:64, t1],
                                     start=True, stop=True, tile_position=(0, 0))
                    nc.tensor.matmul(sAB[:, 512:512 + w], k_t[64:128, t2],
                                     q_t[64:128, t1],
                                     start=True, stop=True, tile_position=(64, 0))
                    pAB = ppool.tile([128, 1024], F16, tag="pAB", bufs=6)
                    if w == 512:
                        nc.scalar.activation(pAB[:], sAB[:], EXP, scale=0.125)
                    else:
                        nc.scalar.activation(pAB[:, 0:w], sAB[:, 0:w],
                                             EXP, scale=0.125)
                        nc.scalar.activation(pAB[:, 512:512 + w],
                                             sAB[:, 512:512 + w],
                                             EXP, scale=0.125)
                    if off >= 0:
                        nc.vector.tensor_mul(pAB[:, 0:128], pAB[:, 0:128],
                                             tri01[:])
                        nc.vector.tensor_mul(pAB[:, 512:640],
                                             pAB[:, 512:640], tri01[:])
                    pvs = bass.ds(512 - w, w)
                    nc.tensor.matmul(pv1[:, pvs], vst[i][:, 2 * j, :],
                                     pAB[:, 0:w],
                                     start=(i == 0), stop=(i == nt2 - 1))
                    nc.tensor.matmul(pv2[:, pvs], vst[i][:, 2 * j + 1, :],
                                     pAB[:, 512:512 + w],
                                     start=(i == 0), stop=(i == nt2 - 1))
                    # drip one filler job per tile into the PE gaps of the
                    # ACT-gated attention stream
                    if pending:
                        fn, args = pending.pop(0)
                        fn(*args)
                tq = bass.ds(blk * 512, 512)
                for h, pv in ((0, pv1), (1, pv2)):
                    rs_sb = ppool.tile([1, 512], F32, tag="rs_sb", bufs=3)
                    nc.vector.tensor_copy(rs_sb[:], pv[D:D + 1, :])
                    rec = ppool.tile([1, 512], F32, tag="rec", bufs=3)
                    nc.vector.reciprocal_approx_fast(rec[:], rs_sb[:])
                    rb = ppool.tile([64, 512], F32, tag="rb", bufs=3)
                    nc.gpsimd.partition_broadcast(rb[:], rec[:])
                    nc.vector.tensor_mul(outT[j][h * 64:(h + 1) * 64, tq],
                                         pv[0:D, :], rb[:])
                if j == 0 and blk > 0:
                    # previous block's exchange goes on the gpsimd queue only
                    # AFTER this pair's norm broadcasts, so they never stall
                    # behind the collective
                    emit_exchange(blk - 1)
                    pending.extend(
                        (proj_job, (blk - 1, mt)) for mt in range(4))
            while pending:
                fn, args = pending.pop(0)
                fn(*args)
        emit_exchange(NT1 - 1)
        for mt in range(4):
            proj_job(NT1 - 1, mt)

    nc.compile()
    _cached['nc'] = nc
    return nc


def make_in_maps(x, w_qkv, b_qkv, w_proj, b_proj):
    x = np.asarray(x, dtype=np.float32)
    w_qkv = np.asarray(w_qkv, dtype=np.float32)
    b_qkv = np.asarray(b_qkv, dtype=np.float32)
    w_proj = np.asarray(w_proj, dtype=np.float32)
    b_proj = np.asarray(b_proj, dtype=np.float32)

    in_maps = []
    for c in range(8):
        b, g = c // 2, c % 2
        heads = list(range(g * HL, (g + 1) * HL))
        # paired column order: chunk j = [q(h_{2j}) | q(h_{2j+1})], then k chunks
        qcols, kcols = [], []
        for j in range(NP):
            for h in (heads[2 * j], heads[2 * j + 1]):
                qcols.extend(range(h * D, (h + 1) * D))
                kcols.extend(range(C + h * D, C + (h + 1) * D))
        vcols = [2 * C + h * D + d for h in heads for d in range(D)]
        qk_idx = np.array(qcols + kcols)
        v_idx = np.array(vcols)

        in_maps.append({
            "xT": np.ascontiguousarray(x[b].T.astype(np.float16)),
            "w_qk": np.ascontiguousarray(w_qkv[:, qk_idx].astype(np.float16)),
            "w_v": np.ascontiguousarray(w_qkv[:, v_idx].astype(np.float16)),
            "b_qk": np.ascontiguousarray(b_qkv[qk_idx][:, None]),
            "b_v": np.ascontiguousarray(b_qkv[v_idx][None, :].astype(np.float16)),
            "w_proj": np.ascontiguousarray(
                w_proj[:, g * 512:(g + 1) * 512].astype(np.float16)),
            "b_proj": np.ascontiguousarray(
                b_proj[None, g * 512:(g + 1) * 512].astype(np.float16)),
        })
    return in_maps


def run(inputs, trace=False):
    if trace:
        install_profile_hook()
    nc = build_kernel()
    in_maps = make_in_maps(**inputs)
    res = run_bass_kernel_spmd(nc, in_maps, list(range(8)), trace=trace)
    out = np.empty((B, T, C), dtype=np.float32)
    for c in range(8):
        b, g = c // 2, c % 2
        out[b, :, g * 512:(g + 1) * 512] = res.results[c]["y_own"]
    return out, res


def kernel(**inputs) -> np.ndarray:
    out, _ = run(inputs, trace=False)
    return out
